# revision 16
# baseline (speedup 1.0000x reference)
"""BCMP layer (GNN message passing) on 8 Trainium2 NeuronCores.

Math (see harness reference):
    out = (ahat(x@WX) + bhat(bcf@WZ) + ahat(bhat(bcf@Walpha))) / 3
By linearity of ahat, and folding the self-loop term d^2*G/3 in as one
more "edge" message (d^2*G/3 = (d/3) * (d*G)):
    out = (d/3) * [ segsum_dest(Gs[col]) + Gs_self ] + bhat(bcf@WZ)/3
    G   = x@WX + bhat(bcf@Walpha),   Gs = d*G  (bf16 messages)

Three SPMD launches over 8 cores (destination nodes sharded, 12500/core,
nodes packed into 98 windows of 128 slots by descending in-degree):

  Launch 0 (tiny): broadcaster tables T[z] = [dcol_z*(bcf@Walpha)_z,
  (dcol_z/3)*(bcf@WZ)_z] (bf16), plus Tcomb[i] = C1*(T[a_i]+T[i]) for
  the i<m rectangular-eye rows via a one-hot Sel matmul (no gather).
  Host: replicate T rows per node (pure data movement).

  Launch 1: per-window psum = x@WX + I@Ta (two matmuls), message
  GS = d*psum (bf16, one scale op, alternating vector/scalar engines).
  Host: shuffle GS rows into per-core, per-window message layout
  MSG[p, c] = GS[src of c-th in-edge of the node in slot p], with one
  extra self column per node (pure integer indexing + data movement).

  Launch 2: stream MSG sequentially; segment-sum each window by
  PSUM-accumulated matmuls with a constant identity lhsT (two message
  columns per matmul); out = (d/3)*psum + TZZ.  No gather, no one-hot.

All floating point math runs on device; the host only does integer
index manipulation (bincount/argsort/packing) and data movement.
"""

import numpy as np
import ml_dtypes

import concourse.bacc as bacc
import concourse.mybir as mybir
from concourse.tile import TileContext
from concourse.bass_utils import run_bass_kernel_spmd

N = 100000
E = 1600000
M = 1000
D = 128
NCORES = 8
NC = N // NCORES            # 12500 nodes per core
P = 128
NW = NC // P + (1 if NC % P else 0)   # 98 windows per core
SLOTS = NW * P              # 12544 slots per core
MPAD = 1024                 # bc rows padded to 8 tiles
MCH = MPAD // P             # 8 column chunks of the broadcaster table
GRP1 = 7                    # windows per group in launch 1
GRP2 = 7                    # windows per group in launch 2
KCOL = 2                    # message columns per matmul in launch 2
C1 = 2.0 ** -0.5

F32 = mybir.dt.float32
BF16 = mybir.dt.bfloat16
AOP = mybir.AluOpType
ACT = mybir.ActivationFunctionType
BF16NP = ml_dtypes.bfloat16

CORE_IDS = list(range(NCORES))

LAST_RESULTS = []           # test harness hook

_kernel_cache = {}


def _groups(n, size):
    return [(s, min(s + size, n)) for s in range(0, n, size)]


def _build_launch0():
    """Broadcaster tables: T[z] = [dcol*(bcf@WA), (dcol/3)*(bcf@WZ)] bf16,
    and Tcomb[i] = C1*(T[a_i] + T[i]) via Sel one-hot matmuls."""
    nc = bacc.Bacc()
    bcfT = nc.declare_dram_parameter("bcfT", [P, MPAD], F32, isOutput=False)
    WAp = nc.declare_dram_parameter("WA", [P, D], F32, isOutput=False)
    WZp = nc.declare_dram_parameter("WZ", [P, D], F32, isOutput=False)
    dcntp = nc.declare_dram_parameter("dcnt", [P, MCH], F32, isOutput=False)
    # sel[p, zc*MPAD + i] = Sel[zc*128+p, i]
    selp = nc.declare_dram_parameter("selT", [P, MCH * MPAD], BF16,
                                     isOutput=False)
    # outputs in chunk layout: row z=c*128+p -> [p, c*256 : (c+1)*256]
    Tt = nc.declare_dram_parameter("T", [P, MCH * 2 * D], BF16, isOutput=True)
    TC = nc.declare_dram_parameter("TC", [P, MCH * 2 * D], BF16, isOutput=True)

    with TileContext(nc) as tc:
        with (
            tc.tile_pool(name="const", bufs=1) as cpool,
            tc.tile_pool(name="psum", bufs=2, space="PSUM") as ppool,
            tc.tile_pool(name="psumc", bufs=1, space="PSUM") as pcpool,
        ):
            waf = cpool.tile([P, 2 * D], F32)
            nc.sync.dma_start(out=waf[:, 0:D], in_=WAp[:])
            nc.sync.dma_start(out=waf[:, D:2 * D], in_=WZp[:])
            wab = cpool.tile([P, 2 * D], BF16)
            nc.vector.tensor_scalar_mul(wab[:], waf[:], 1.0)
            bcff = cpool.tile([P, MPAD], F32)
            nc.sync.dma_start(out=bcff[:], in_=bcfT[:])
            bcf = cpool.tile([P, MPAD], BF16)
            nc.vector.tensor_scalar_mul(bcf[:], bcff[:], 1.0)
            dcnt = cpool.tile([P, MCH], F32)
            nc.sync.dma_start(out=dcnt[:], in_=dcntp[:])
            sel = cpool.tile([P, MCH * MPAD], BF16)
            nc.sync.dma_start(out=sel[:], in_=selp[:])

            rcc = cpool.tile([P, MCH], F32)
            nc.vector.reciprocal(rcc[:], dcnt[:])
            dcol = cpool.tile([P, MCH], F32)
            nc.scalar.activation(dcol[:], rcc[:], ACT.Sqrt)
            dcol3 = cpool.tile([P, MCH], F32)
            nc.scalar.activation(dcol3[:], rcc[:], ACT.Sqrt, scale=1.0 / 9.0)

            tst = cpool.tile([P, MCH * 2 * D], BF16)   # T staging (and rhs)
            for zc in range(MCH):
                pz = ppool.tile([P, 2 * D], F32, space="PSUM", tag="pz")
                nc.tensor.matmul(
                    out=pz[:, 0:D], lhsT=bcf[:, zc * P:(zc + 1) * P],
                    rhs=wab[:, 0:D], start=True, stop=True,
                )
                nc.tensor.matmul(
                    out=pz[:, D:2 * D], lhsT=bcf[:, zc * P:(zc + 1) * P],
                    rhs=wab[:, D:2 * D], start=True, stop=True,
                )
                nc.vector.tensor_scalar(
                    out=tst[:, zc * 2 * D:zc * 2 * D + D], in0=pz[:, 0:D],
                    scalar1=dcol[:, zc:zc + 1], scalar2=None, op0=AOP.mult,
                )
                nc.vector.tensor_scalar(
                    out=tst[:, zc * 2 * D + D:(zc + 1) * 2 * D],
                    in0=pz[:, D:2 * D],
                    scalar1=dcol3[:, zc:zc + 1], scalar2=None, op0=AOP.mult,
                )
            nc.sync.dma_start(out=Tt[:], in_=tst[:])

            # Tcomb: 4 live psum accumulators, zc outer so each Sel matmul
            # fires as soon as its tst chunk is ready (overlaps the T chain)
            tcst = cpool.tile([P, MCH * 2 * D], BF16)  # Tcomb staging
            HB = MCH // 2
            for half in range(2):
                pcs = [pcpool.tile([P, 2 * D], F32, space="PSUM",
                                   name=f"pc{k}", tag=f"pc{k}")
                       for k in range(HB)]
                for zc in range(MCH):
                    for k in range(HB):
                        ic = half * HB + k
                        nc.tensor.matmul(
                            out=pcs[k][:],
                            lhsT=sel[:, zc * MPAD + ic * P:
                                     zc * MPAD + (ic + 1) * P],
                            rhs=tst[:, zc * 2 * D:(zc + 1) * 2 * D],
                            start=(zc == 0), stop=(zc == MCH - 1),
                        )
                for k in range(HB):
                    ic = half * HB + k
                    u = cpool.tile([P, 2 * D], F32, tag=f"u{ic}")
                    nc.vector.tensor_add(
                        out=u[:], in0=pcs[k][:],
                        in1=tst[:, ic * 2 * D:(ic + 1) * 2 * D],
                    )
                    nc.vector.tensor_scalar_mul(
                        tcst[:, ic * 2 * D:(ic + 1) * 2 * D], u[:], C1,
                    )
            nc.sync.dma_start(out=TC[:], in_=tcst[:])

    nc.compile()
    return nc


def _build_launch1():
    """Node phase: GS = d*(x@WX + Ta), adds on the tensor engine."""
    nc = bacc.Bacc()
    xT = nc.declare_dram_parameter("xT", [P, SLOTS], F32, isOutput=False)
    WXp = nc.declare_dram_parameter("WX", [P, D], F32, isOutput=False)
    tap = nc.declare_dram_parameter("taT", [P, NW * D], BF16, isOutput=False)
    degp = nc.declare_dram_parameter("deg", [P, NW], F32, isOutput=False)
    identp = nc.declare_dram_parameter("ident", [P, D], BF16, isOutput=False)
    GS = nc.declare_dram_parameter("GS", [P, NW * D], BF16, isOutput=True)

    groups = _groups(NW, GRP1)
    with TileContext(nc) as tc:
        with (
            tc.tile_pool(name="const", bufs=1) as cpool,
            tc.tile_pool(name="xin", bufs=3) as xpool,
            tc.tile_pool(name="ta", bufs=2) as tpool,
            tc.tile_pool(name="stage", bufs=2) as spool,
            tc.tile_pool(name="psum", bufs=4, space="PSUM") as ppool,
        ):
            wxf = cpool.tile([P, D], F32)
            nc.sync.dma_start(out=wxf[:], in_=WXp[:])
            wx = cpool.tile([P, D], BF16)
            nc.vector.tensor_scalar_mul(wx[:], wxf[:], 1.0)
            ident = cpool.tile([P, D], BF16)
            nc.sync.dma_start(out=ident[:], in_=identp[:])
            deg = cpool.tile([P, NW], F32)
            nc.sync.dma_start(out=deg[:], in_=degp[:])
            rec = cpool.tile([P, NW], F32)
            nc.vector.reciprocal(rec[:], deg[:])
            dsb = cpool.tile([P, NW], F32)
            nc.scalar.activation(dsb[:], rec[:], ACT.Sqrt)       # d

            for lo, hi in groups:
                gw = hi - lo
                xg = xpool.tile([P, GRP1 * P], F32, tag="xg")
                nc.sync.dma_start(
                    out=xg[:, :gw * P], in_=xT[:, lo * P:hi * P],
                )
                xb = xpool.tile([P, GRP1 * P], BF16, tag="xb")
                nc.vector.tensor_scalar_mul(xb[:, :gw * P], xg[:, :gw * P],
                                            1.0)
                tat = tpool.tile([P, GRP1 * D], BF16, tag="tat")
                nc.sync.dma_start(
                    out=tat[:, :gw * D], in_=tap[:, lo * D:hi * D],
                )
                gst = spool.tile([P, GRP1 * D], BF16, tag="gst")
                for wl in range(gw):
                    j = lo + wl
                    ps = ppool.tile([P, D], F32, space="PSUM", tag="ps")
                    nc.tensor.matmul(
                        out=ps[:], lhsT=xb[:, wl * P:(wl + 1) * P], rhs=wx[:],
                        start=True, stop=False,
                    )
                    nc.tensor.matmul(
                        out=ps[:], lhsT=ident[:],
                        rhs=tat[:, wl * D:(wl + 1) * D],
                        start=False, stop=True,
                    )
                    # GS = d*(x@WX + Ta): alternate vector/scalar engines
                    if wl % 2 == 0:
                        nc.scalar.activation(
                            gst[:, wl * D:(wl + 1) * D], ps[:], ACT.Copy,
                            scale=dsb[:, j:j + 1],
                        )
                    else:
                        nc.vector.tensor_scalar(
                            out=gst[:, wl * D:(wl + 1) * D], in0=ps[:],
                            scalar1=dsb[:, j:j + 1], scalar2=None,
                            op0=AOP.mult,
                        )
                nc.sync.dma_start(
                    out=GS[:, lo * D:hi * D], in_=gst[:, :gw * D],
                )

    nc.compile()
    return nc


def _build_launch2(cws):
    """Edge phase: psum_w = sum_c MSG[:, c] (incl. self column);
    out = (d/3)*psum + TZZ.  cws = per-window column counts."""
    cws = list(cws)
    CT = sum(cws)
    off = np.concatenate([[0], np.cumsum(cws)])
    groups = _groups(NW, GRP2)
    gcmax = max(int(off[hi] - off[lo]) for lo, hi in groups)

    nc = bacc.Bacc()
    MSGp = nc.declare_dram_parameter("MSG", [P, CT * D], BF16, isOutput=False)
    Rwp = nc.declare_dram_parameter("Rw", [P, NW * D], BF16, isOutput=False)
    degp = nc.declare_dram_parameter("degw", [P, NW], F32, isOutput=False)
    identp = nc.declare_dram_parameter("ident", [P, D], BF16, isOutput=False)
    OUT = nc.declare_dram_parameter("OUT", [P, NW * D], BF16, isOutput=True)

    with TileContext(nc) as tc:
        with (
            tc.tile_pool(name="const", bufs=1) as cpool,
            tc.tile_pool(name="msg", bufs=2) as mpool,
            tc.tile_pool(name="rw", bufs=2) as rpool,
            tc.tile_pool(name="fin", bufs=3) as fpool,
            tc.tile_pool(name="out", bufs=2) as opool,
            tc.tile_pool(name="psum", bufs=4, space="PSUM") as ppool,
        ):
            ident = cpool.tile([P, D], BF16)
            nc.sync.dma_start(out=ident[:], in_=identp[:])
            degw = cpool.tile([P, NW], F32)
            nc.sync.dma_start(out=degw[:], in_=degp[:])
            rec = cpool.tile([P, NW], F32)
            nc.vector.reciprocal(rec[:], degw[:])
            dsc = cpool.tile([P, NW], F32)
            nc.scalar.activation(dsc[:], rec[:], ACT.Sqrt, scale=1.0 / 9.0)

            # descending window size: the last group has the least compute,
            # minimizing the post-stream drain (span ~= dma + last compute)
            for lo, hi in groups:
                gofflo, goffhi = int(off[lo]), int(off[hi])
                gc = goffhi - gofflo
                msg = mpool.tile([P, gcmax * D], BF16, tag="msg")
                nc.sync.dma_start(
                    out=msg[:, :gc * D],
                    in_=MSGp[:, gofflo * D:goffhi * D],
                )
                rw = rpool.tile([P, GRP2 * D], BF16, tag="rw")
                nc.sync.dma_start(
                    out=rw[:, :(hi - lo) * D],
                    in_=Rwp[:, lo * D:hi * D],
                )
                ost = opool.tile([P, GRP2 * D], BF16, tag="ost")
                for wl in range(hi - lo):
                    w = lo + wl
                    cw = cws[w]
                    base = (int(off[w]) - gofflo) * D
                    ps = ppool.tile([P, KCOL * D], F32, space="PSUM")
                    npair = cw // KCOL
                    odd = cw % KCOL
                    for b in range(npair):
                        nc.tensor.matmul(
                            out=ps[:], lhsT=ident[:],
                            rhs=msg[:, base + b * KCOL * D:
                                    base + (b + 1) * KCOL * D],
                            start=(b == 0), stop=(b == npair - 1 and not odd),
                        )
                    if odd:  # trailing single column into the left half
                        nc.tensor.matmul(
                            out=ps[:, 0:D], lhsT=ident[:],
                            rhs=msg[:, base + npair * KCOL * D:
                                    base + (npair * KCOL + 1) * D],
                            start=False, stop=True, skip_group_check=True,
                        )
                    # single scaled PSUM->SBUF copy (one PSUM operand)
                    u2 = fpool.tile([P, KCOL * D], F32, tag="u2")
                    nc.scalar.activation(
                        u2[:], ps[:], ACT.Copy, scale=dsc[:, w:w + 1],
                    )
                    t = fpool.tile([P, D], F32, tag="t")
                    nc.vector.tensor_add(
                        out=t[:], in0=u2[:, 0:D], in1=u2[:, D:2 * D],
                    )
                    nc.vector.tensor_add(
                        out=ost[:, wl * D:(wl + 1) * D], in0=t[:],
                        in1=rw[:, wl * D:(wl + 1) * D],
                    )
                nc.sync.dma_start(
                    out=OUT[:, lo * D:hi * D], in_=ost[:, :(hi - lo) * D],
                )

    nc.compile()
    return nc


def _get_kernels(cw_key):
    if "l0" not in _kernel_cache:
        _kernel_cache["l0"] = _build_launch0()
    if "l1" not in _kernel_cache:
        _kernel_cache["l1"] = _build_launch1()
    if ("l2", cw_key) not in _kernel_cache:
        _kernel_cache[("l2", cw_key)] = _build_launch2(cw_key)
    return (_kernel_cache["l0"], _kernel_cache["l1"],
            _kernel_cache[("l2", cw_key)])


def _pack_slots(vec, pad_value, ncols):
    """[values] -> [P, ncols] with flat index col*128+p."""
    tmp = np.full(ncols * P, pad_value, dtype=vec.dtype)
    tmp[: len(vec)] = vec
    return np.ascontiguousarray(tmp.reshape(ncols, P).T)


def kernel(x, edge_index, bc_feature, bc_assignment, WX, WZ, Walpha):
    x = np.asarray(x, dtype=np.float32)
    edge_index = np.asarray(edge_index)
    bc_feature = np.asarray(bc_feature, dtype=np.float32)
    bc_assignment = np.asarray(bc_assignment)
    WX = np.asarray(WX, dtype=np.float32)
    WZ = np.asarray(WZ, dtype=np.float32)
    Walpha = np.asarray(Walpha, dtype=np.float32)

    row = edge_index[0].astype(np.int64)   # dest (aggregation target)
    col = edge_index[1].astype(np.int64)   # src  (message provider)
    assign = bc_assignment.astype(np.int64)

    deg = (np.bincount(col, minlength=N) + 1).astype(np.float32)  # for d
    cnt = (np.bincount(assign, minlength=M) + 1).astype(np.float32)
    indeg = np.bincount(row, minlength=N).astype(np.int64)

    order_e = np.argsort(row, kind="stable")
    row_s = row[order_e]
    col_s = col[order_e]
    bounds = np.searchsorted(row_s, np.arange(N + 1))

    # Per-core degree-sorted window packing (slot = rank in desc in-degree).
    perms = []       # perm[slot rank] = global node id
    for c in range(NCORES):
        ideg = indeg[c * NC:(c + 1) * NC]
        order_n = np.argsort(-ideg, kind="stable")
        perms.append(c * NC + order_n)
    # Shared per-window column counts (max over cores, +1 self, KCOL-aligned).
    cws = np.zeros(NW, dtype=np.int64)
    for c in range(NCORES):
        s = indeg[perms[c]]
        pad = np.zeros(SLOTS, dtype=np.int64)
        pad[:NC] = s
        cws = np.maximum(cws, pad.reshape(NW, P).max(axis=1))
    cws = cws + 1                                    # self column
    cw_key = tuple(int(v) for v in cws)
    off = np.concatenate([[0], np.cumsum(cws)])
    CT = int(off[-1])

    nc0, nc1, nc2 = _get_kernels(cw_key)

    # ---------------- launch 0: broadcaster tables ----------------
    bcfT = np.zeros((P, MPAD), dtype=np.float32)
    bcfT[:, :M] = bc_feature.T
    a_pad = np.zeros(MPAD, dtype=np.int64)
    a_pad[:M] = assign[:M]
    selT = np.zeros((MPAD, MPAD), dtype=BF16NP)
    selT[a_pad[:M], np.arange(M)] = 1.0
    in0 = {
        "bcfT": bcfT, "WA": Walpha, "WZ": WZ,
        "dcnt": _pack_slots(cnt, np.float32(1.0), MCH),
        "selT": np.ascontiguousarray(
            selT.reshape(MCH, P, MPAD).transpose(1, 0, 2)
            .reshape(P, MCH * MPAD)
        ),
    }
    res0 = run_bass_kernel_spmd(nc0, [in0] * NCORES, core_ids=CORE_IDS)
    LAST_RESULTS.clear()
    LAST_RESULTS.append(res0)

    # chunk layout [128, 8*256]: row i lives at [i%128, (i//128)*256:...]
    def _unchunk(arr):
        return np.ascontiguousarray(
            arr.reshape(P, MCH, 2 * D).transpose(1, 0, 2).reshape(MPAD, 2 * D)
        )

    T_np = _unchunk(np.asarray(res0.results[0]["T"]))
    Tcomb = _unchunk(np.asarray(res0.results[0]["TC"]))

    iden = np.zeros((P, D), dtype=BF16NP)
    np.fill_diagonal(iden, 1.0)

    # ---------------- launch 1: node phase ----------------
    in_maps1 = []
    treps = []
    for c in range(NCORES):
        perm = perms[c]
        xpad = np.zeros((SLOTS, D), dtype=np.float32)
        xpad[:NC] = x[perm]
        trep = np.zeros((SLOTS, 2 * D), dtype=BF16NP)
        trep[:NC] = T_np[assign[perm]]
        eye_mask = perm < M
        if eye_mask.any():
            ranks = np.nonzero(eye_mask)[0]
            trep[ranks] = Tcomb[perm[ranks]]
        treps.append(trep)
        degv = np.ones(SLOTS, dtype=np.float32)
        degv[:NC] = deg[perm]
        in_maps1.append({
            "xT": np.ascontiguousarray(xpad.T),
            "WX": WX,
            "taT": np.ascontiguousarray(
                trep[:, :D].reshape(NW, P, D).transpose(1, 0, 2)
                .reshape(P, NW * D)
            ),
            "deg": np.ascontiguousarray(degv.reshape(NW, P).T),
            "ident": iden,
        })
    res1 = run_bass_kernel_spmd(nc1, in_maps1, core_ids=CORE_IDS)
    LAST_RESULTS.append(res1)

    # GS[p, w*D:] holds node perm[w*128+p]; restore node order globally.
    GSe = np.zeros((N + 1, D), dtype=BF16NP)   # +1 zero row for padding
    for c in range(NCORES):
        gs = np.asarray(res1.results[c]["GS"])       # [P, NW*D]
        gs = gs.reshape(P, NW, D).transpose(1, 0, 2).reshape(SLOTS, D)
        GSe[perms[c]] = gs[:NC]

    # ---------------- launch 2: edge phase ----------------
    in_maps2 = []
    for c in range(NCORES):
        perm = perms[c]
        slotof = np.empty(NC, dtype=np.int64)
        slotof[perm - c * NC] = np.arange(NC)
        lo, hi = bounds[c * NC], bounds[(c + 1) * NC]
        rnk = slotof[row_s[lo:hi] - c * NC]
        kth = np.arange(lo, hi) - bounds[row_s[lo:hi]]
        srcidx = np.full((P, CT), N, dtype=np.int64)
        # self column first, then the in-edges
        allrnk = np.arange(NC)
        srcidx[allrnk & 127, off[allrnk >> 7]] = perm
        srcidx[rnk & 127, off[rnk >> 7] + 1 + kth] = col_s[lo:hi]
        MSG = GSe[srcidx.ravel()].reshape(P, CT * D)
        degv = np.ones(SLOTS, dtype=np.float32)
        degv[:NC] = deg[perm]
        tzz = treps[c][:, D:]
        in_maps2.append({
            "MSG": np.ascontiguousarray(MSG),
            "Rw": np.ascontiguousarray(
                tzz.reshape(NW, P, D).transpose(1, 0, 2).reshape(P, NW * D)
            ),
            "degw": np.ascontiguousarray(degv.reshape(NW, P).T),
            "ident": iden,
        })
    res2 = run_bass_kernel_spmd(nc2, in_maps2, core_ids=CORE_IDS)
    LAST_RESULTS.append(res2)

    out = np.empty((N, D), dtype=np.float32)
    for c in range(NCORES):
        o = np.asarray(res2.results[c]["OUT"]).astype(np.float32)
        o = o.reshape(P, NW, D).transpose(1, 0, 2).reshape(SLOTS, D)
        out[perms[c]] = o[:NC]
    return out


# revision 23
# speedup vs baseline: 1.0232x; 1.0232x over previous
"""BCMP layer (GNN message passing) on 8 Trainium2 NeuronCores.

Math (see harness reference):
    out = (ahat(x@WX) + bhat(bcf@WZ) + ahat(bhat(bcf@Walpha))) / 3
By linearity of ahat, and folding the self-loop term d^2*G/3 in as one
more "edge" message (d^2*G/3 = (d/3) * (d*G)):
    out = (d/3) * [ segsum_dest(Gs[col]) + Gs_self ] + bhat(bcf@WZ)/3
    G   = x@WX + bhat(bcf@Walpha),   Gs = d*G  (bf16 messages)

Three SPMD launches over 8 cores (destination nodes sharded, 12500/core,
nodes packed into 98 windows of 128 slots by descending in-degree):

  Launch 0 (tiny): broadcaster tables T[z] = [dcol_z*(bcf@Walpha)_z,
  (dcol_z/3)*(bcf@WZ)_z] (bf16), plus Tcomb[i] = C1*(T[a_i]+T[i]) for
  the i<m rectangular-eye rows via a one-hot Sel matmul (no gather).
  Host: replicate T rows per node (pure data movement).

  Launch 1: per-window psum = x@WX + I@Ta (two matmuls), message
  GS = d*psum (bf16, one scale op, alternating vector/scalar engines).
  Host: shuffle GS rows into per-core, per-window message layout
  MSG[p, c] = GS[src of c-th in-edge of the node in slot p], with one
  extra self column per node (pure integer indexing + data movement).

  Launch 2: stream MSG sequentially; segment-sum each window by
  PSUM-accumulated matmuls with a constant identity lhsT (two message
  columns per matmul); out = (d/3)*psum + TZZ.  No gather, no one-hot.

All floating point math runs on device; the host only does integer
index manipulation (bincount/argsort/packing) and data movement.
"""

import numpy as np
import ml_dtypes

import concourse.bacc as bacc
import concourse.mybir as mybir
from concourse.tile import TileContext
from concourse.bass_utils import run_bass_kernel_spmd

N = 100000
E = 1600000
M = 1000
D = 128
NCORES = 8
NC = N // NCORES            # 12500 nodes per core
P = 128
NW = NC // P + (1 if NC % P else 0)   # 98 windows per core
SLOTS = NW * P              # 12544 slots per core
MPAD = 1024                 # bc rows padded to 8 tiles
MCH = MPAD // P             # 8 column chunks of the broadcaster table
GRP1 = 14                   # windows per group in launch 1
GRP2 = 7                    # windows per group in launch 2
KCOL = 2                    # message columns per matmul in launch 2
C1 = 2.0 ** -0.5

F32 = mybir.dt.float32
BF16 = mybir.dt.bfloat16
AOP = mybir.AluOpType
ACT = mybir.ActivationFunctionType
BF16NP = ml_dtypes.bfloat16

CORE_IDS = list(range(NCORES))

LAST_RESULTS = []           # test harness hook

_kernel_cache = {}


def _groups(n, size):
    return [(s, min(s + size, n)) for s in range(0, n, size)]


def _build_launch0():
    """Broadcaster tables: T[z] = [dcol*(bcf@WA), (dcol/3)*(bcf@WZ)] bf16,
    and Tcomb[i] = C1*(T[a_i] + T[i]) via Sel one-hot matmuls."""
    nc = bacc.Bacc()
    bcfT = nc.declare_dram_parameter("bcfT", [P, MPAD], F32, isOutput=False)
    WAp = nc.declare_dram_parameter("WA", [P, D], F32, isOutput=False)
    WZp = nc.declare_dram_parameter("WZ", [P, D], F32, isOutput=False)
    dcntp = nc.declare_dram_parameter("dcnt", [P, MCH], F32, isOutput=False)
    # sel[p, zc*MPAD + i] = Sel[zc*128+p, i]
    selp = nc.declare_dram_parameter("selT", [P, MCH * MPAD], BF16,
                                     isOutput=False)
    # outputs in chunk layout: row z=c*128+p -> [p, c*256 : (c+1)*256]
    Tt = nc.declare_dram_parameter("T", [P, MCH * 2 * D], BF16, isOutput=True)
    TC = nc.declare_dram_parameter("TC", [P, MCH * 2 * D], BF16, isOutput=True)

    with TileContext(nc) as tc:
        with (
            tc.tile_pool(name="const", bufs=1) as cpool,
            tc.tile_pool(name="psum", bufs=2, space="PSUM") as ppool,
            tc.tile_pool(name="psumc", bufs=1, space="PSUM") as pcpool,
        ):
            waf = cpool.tile([P, 2 * D], F32)
            nc.sync.dma_start(out=waf[:, 0:D], in_=WAp[:])
            nc.sync.dma_start(out=waf[:, D:2 * D], in_=WZp[:])
            wab = cpool.tile([P, 2 * D], BF16)
            nc.vector.tensor_scalar_mul(wab[:], waf[:], 1.0)
            bcff = cpool.tile([P, MPAD], F32)
            nc.sync.dma_start(out=bcff[:], in_=bcfT[:])
            bcf = cpool.tile([P, MPAD], BF16)
            nc.vector.tensor_scalar_mul(bcf[:], bcff[:], 1.0)
            dcnt = cpool.tile([P, MCH], F32)
            nc.sync.dma_start(out=dcnt[:], in_=dcntp[:])
            sel = cpool.tile([P, MCH * MPAD], BF16)
            nc.sync.dma_start(out=sel[:], in_=selp[:])

            rcc = cpool.tile([P, MCH], F32)
            nc.vector.reciprocal(rcc[:], dcnt[:])
            dcol = cpool.tile([P, MCH], F32)
            nc.scalar.activation(dcol[:], rcc[:], ACT.Sqrt)
            dcol3 = cpool.tile([P, MCH], F32)
            nc.scalar.activation(dcol3[:], rcc[:], ACT.Sqrt, scale=1.0 / 9.0)

            tst = cpool.tile([P, MCH * 2 * D], BF16)   # T staging (and rhs)
            for zc in range(MCH):
                pz = ppool.tile([P, 2 * D], F32, space="PSUM", tag="pz")
                nc.tensor.matmul(
                    out=pz[:, 0:D], lhsT=bcf[:, zc * P:(zc + 1) * P],
                    rhs=wab[:, 0:D], start=True, stop=True,
                )
                nc.tensor.matmul(
                    out=pz[:, D:2 * D], lhsT=bcf[:, zc * P:(zc + 1) * P],
                    rhs=wab[:, D:2 * D], start=True, stop=True,
                )
                nc.vector.tensor_scalar(
                    out=tst[:, zc * 2 * D:zc * 2 * D + D], in0=pz[:, 0:D],
                    scalar1=dcol[:, zc:zc + 1], scalar2=None, op0=AOP.mult,
                )
                nc.vector.tensor_scalar(
                    out=tst[:, zc * 2 * D + D:(zc + 1) * 2 * D],
                    in0=pz[:, D:2 * D],
                    scalar1=dcol3[:, zc:zc + 1], scalar2=None, op0=AOP.mult,
                )
            nc.sync.dma_start(out=Tt[:], in_=tst[:])

            # Tcomb: 4 live psum accumulators, zc outer so each Sel matmul
            # fires as soon as its tst chunk is ready (overlaps the T chain)
            tcst = cpool.tile([P, MCH * 2 * D], BF16)  # Tcomb staging
            HB = MCH // 2
            for half in range(2):
                pcs = [pcpool.tile([P, 2 * D], F32, space="PSUM",
                                   name=f"pc{k}", tag=f"pc{k}")
                       for k in range(HB)]
                for zc in range(MCH):
                    for k in range(HB):
                        ic = half * HB + k
                        nc.tensor.matmul(
                            out=pcs[k][:],
                            lhsT=sel[:, zc * MPAD + ic * P:
                                     zc * MPAD + (ic + 1) * P],
                            rhs=tst[:, zc * 2 * D:(zc + 1) * 2 * D],
                            start=(zc == 0), stop=(zc == MCH - 1),
                        )
                for k in range(HB):
                    ic = half * HB + k
                    u = cpool.tile([P, 2 * D], F32, tag=f"u{ic}")
                    nc.vector.tensor_add(
                        out=u[:], in0=pcs[k][:],
                        in1=tst[:, ic * 2 * D:(ic + 1) * 2 * D],
                    )
                    nc.vector.tensor_scalar_mul(
                        tcst[:, ic * 2 * D:(ic + 1) * 2 * D], u[:], C1,
                    )
            nc.sync.dma_start(out=TC[:], in_=tcst[:])

    nc.compile()
    return nc


def _build_launch1():
    """Node phase: GS = d*(x@WX + Ta), adds on the tensor engine."""
    nc = bacc.Bacc()
    xT = nc.declare_dram_parameter("xT", [P, SLOTS], F32, isOutput=False)
    WXp = nc.declare_dram_parameter("WX", [P, D], F32, isOutput=False)
    tap = nc.declare_dram_parameter("taT", [P, NW * D], BF16, isOutput=False)
    degp = nc.declare_dram_parameter("deg", [P, NW], F32, isOutput=False)
    identp = nc.declare_dram_parameter("ident", [P, D], BF16, isOutput=False)
    GS = nc.declare_dram_parameter("GS", [P, NW * D], BF16, isOutput=True)

    groups = _groups(NW, GRP1)
    with TileContext(nc) as tc:
        with (
            tc.tile_pool(name="const", bufs=1) as cpool,
            tc.tile_pool(name="xin", bufs=3) as xpool,
            tc.tile_pool(name="stage", bufs=2) as spool,
            tc.tile_pool(name="psum", bufs=4, space="PSUM") as ppool,
        ):
            wxf = cpool.tile([P, D], F32)
            nc.sync.dma_start(out=wxf[:], in_=WXp[:])
            wx = cpool.tile([P, D], BF16)
            nc.vector.tensor_scalar_mul(wx[:], wxf[:], 1.0)
            ident = cpool.tile([P, D], BF16)
            nc.sync.dma_start(out=ident[:], in_=identp[:])
            deg = cpool.tile([P, NW], F32)
            nc.sync.dma_start(out=deg[:], in_=degp[:])
            # whole Ta plane prefetched on the scalar HWDGE queue, so the
            # sync queue only carries the x chunks and GS writes
            tat = cpool.tile([P, NW * D], BF16)
            nc.scalar.dma_start(out=tat[:], in_=tap[:])
            rec = cpool.tile([P, NW], F32)
            nc.vector.reciprocal(rec[:], deg[:])
            dsb = cpool.tile([P, NW], F32)
            nc.scalar.activation(dsb[:], rec[:], ACT.Sqrt)       # d

            for lo, hi in groups:
                gw = hi - lo
                xg = xpool.tile([P, GRP1 * P], F32, tag="xg")
                nc.sync.dma_start(
                    out=xg[:, :gw * P], in_=xT[:, lo * P:hi * P],
                )
                xb = xpool.tile([P, GRP1 * P], BF16, tag="xb")
                nc.vector.tensor_scalar_mul(xb[:, :gw * P], xg[:, :gw * P],
                                            1.0)
                gst = spool.tile([P, GRP1 * D], BF16, tag="gst")
                for wl in range(gw):
                    j = lo + wl
                    ps = ppool.tile([P, D], F32, space="PSUM", tag="ps")
                    nc.tensor.matmul(
                        out=ps[:], lhsT=xb[:, wl * P:(wl + 1) * P], rhs=wx[:],
                        start=True, stop=False,
                    )
                    nc.tensor.matmul(
                        out=ps[:], lhsT=ident[:],
                        rhs=tat[:, j * D:(j + 1) * D],
                        start=False, stop=True,
                    )
                    # GS = d*(x@WX + Ta): alternate vector/scalar engines
                    if wl % 2 == 0:
                        nc.scalar.activation(
                            gst[:, wl * D:(wl + 1) * D], ps[:], ACT.Copy,
                            scale=dsb[:, j:j + 1],
                        )
                    else:
                        nc.vector.tensor_scalar(
                            out=gst[:, wl * D:(wl + 1) * D], in0=ps[:],
                            scalar1=dsb[:, j:j + 1], scalar2=None,
                            op0=AOP.mult,
                        )
                nc.sync.dma_start(
                    out=GS[:, lo * D:hi * D], in_=gst[:, :gw * D],
                )

    nc.compile()
    return nc


def _build_launch2(cws):
    """Edge phase: psum_w = sum_c MSG[:, c] (incl. self column);
    out = (d/3)*psum + TZZ.  cws = per-window column counts."""
    cws = list(cws)
    CT = sum(cws)
    off = np.concatenate([[0], np.cumsum(cws)])
    groups = _groups(NW, GRP2)
    gcmax = max(int(off[hi] - off[lo]) for lo, hi in groups)

    nc = bacc.Bacc()
    MSGp = nc.declare_dram_parameter("MSG", [P, CT * D], BF16, isOutput=False)
    Rwp = nc.declare_dram_parameter("Rw", [P, NW * D], BF16, isOutput=False)
    degp = nc.declare_dram_parameter("degw", [P, NW], F32, isOutput=False)
    identp = nc.declare_dram_parameter("ident", [P, D], BF16, isOutput=False)
    OUT = nc.declare_dram_parameter("OUT", [P, NW * D], BF16, isOutput=True)

    with TileContext(nc) as tc:
        with (
            tc.tile_pool(name="const", bufs=1) as cpool,
            tc.tile_pool(name="msg", bufs=2) as mpool,
            tc.tile_pool(name="fin", bufs=3) as fpool,
            tc.tile_pool(name="out", bufs=2) as opool,
            tc.tile_pool(name="psum", bufs=4, space="PSUM") as ppool,
        ):
            ident = cpool.tile([P, D], BF16)
            nc.sync.dma_start(out=ident[:], in_=identp[:])
            degw = cpool.tile([P, NW], F32)
            nc.sync.dma_start(out=degw[:], in_=degp[:])
            rwall = cpool.tile([P, NW * D], BF16)
            nc.scalar.dma_start(out=rwall[:], in_=Rwp[:])
            rec = cpool.tile([P, NW], F32)
            nc.vector.reciprocal(rec[:], degw[:])
            dsc = cpool.tile([P, NW], F32)
            nc.scalar.activation(dsc[:], rec[:], ACT.Sqrt, scale=1.0 / 9.0)

            # descending window size: the last group has the least compute,
            # minimizing the post-stream drain (span ~= dma + last compute)
            for lo, hi in groups:
                gofflo, goffhi = int(off[lo]), int(off[hi])
                gc = goffhi - gofflo
                msg = mpool.tile([P, gcmax * D], BF16, tag="msg")
                nc.sync.dma_start(
                    out=msg[:, :gc * D],
                    in_=MSGp[:, gofflo * D:goffhi * D],
                )
                ost = opool.tile([P, GRP2 * D], BF16, tag="ost")
                for wl in range(hi - lo):
                    w = lo + wl
                    cw = cws[w]
                    base = (int(off[w]) - gofflo) * D
                    ps = ppool.tile([P, KCOL * D], F32, space="PSUM")
                    npair = cw // KCOL
                    odd = cw % KCOL
                    for b in range(npair):
                        nc.tensor.matmul(
                            out=ps[:], lhsT=ident[:],
                            rhs=msg[:, base + b * KCOL * D:
                                    base + (b + 1) * KCOL * D],
                            start=(b == 0), stop=(b == npair - 1 and not odd),
                        )
                    if odd:  # trailing single column into the left half
                        nc.tensor.matmul(
                            out=ps[:, 0:D], lhsT=ident[:],
                            rhs=msg[:, base + npair * KCOL * D:
                                    base + (npair * KCOL + 1) * D],
                            start=False, stop=True, skip_group_check=True,
                        )
                    # single scaled PSUM->SBUF copy (one PSUM operand)
                    u2 = fpool.tile([P, KCOL * D], F32, tag="u2")
                    nc.scalar.activation(
                        u2[:], ps[:], ACT.Copy, scale=dsc[:, w:w + 1],
                    )
                    t = fpool.tile([P, D], F32, tag="t")
                    nc.vector.tensor_add(
                        out=t[:], in0=u2[:, 0:D], in1=u2[:, D:2 * D],
                    )
                    nc.vector.tensor_add(
                        out=ost[:, wl * D:(wl + 1) * D], in0=t[:],
                        in1=rwall[:, w * D:(w + 1) * D],
                    )
                nc.sync.dma_start(
                    out=OUT[:, lo * D:hi * D], in_=ost[:, :(hi - lo) * D],
                )

    nc.compile()
    return nc


def _get_kernels(cw_key):
    if "l0" not in _kernel_cache:
        _kernel_cache["l0"] = _build_launch0()
    if "l1" not in _kernel_cache:
        _kernel_cache["l1"] = _build_launch1()
    if ("l2", cw_key) not in _kernel_cache:
        _kernel_cache[("l2", cw_key)] = _build_launch2(cw_key)
    return (_kernel_cache["l0"], _kernel_cache["l1"],
            _kernel_cache[("l2", cw_key)])


def _pack_slots(vec, pad_value, ncols):
    """[values] -> [P, ncols] with flat index col*128+p."""
    tmp = np.full(ncols * P, pad_value, dtype=vec.dtype)
    tmp[: len(vec)] = vec
    return np.ascontiguousarray(tmp.reshape(ncols, P).T)


def kernel(x, edge_index, bc_feature, bc_assignment, WX, WZ, Walpha):
    x = np.asarray(x, dtype=np.float32)
    edge_index = np.asarray(edge_index)
    bc_feature = np.asarray(bc_feature, dtype=np.float32)
    bc_assignment = np.asarray(bc_assignment)
    WX = np.asarray(WX, dtype=np.float32)
    WZ = np.asarray(WZ, dtype=np.float32)
    Walpha = np.asarray(Walpha, dtype=np.float32)

    row = edge_index[0].astype(np.int64)   # dest (aggregation target)
    col = edge_index[1].astype(np.int64)   # src  (message provider)
    assign = bc_assignment.astype(np.int64)

    deg = (np.bincount(col, minlength=N) + 1).astype(np.float32)  # for d
    cnt = (np.bincount(assign, minlength=M) + 1).astype(np.float32)
    indeg = np.bincount(row, minlength=N).astype(np.int64)

    order_e = np.argsort(row, kind="stable")
    row_s = row[order_e]
    col_s = col[order_e]
    bounds = np.searchsorted(row_s, np.arange(N + 1))

    # Per-core degree-sorted window packing (slot = rank in desc in-degree).
    perms = []       # perm[slot rank] = global node id
    for c in range(NCORES):
        ideg = indeg[c * NC:(c + 1) * NC]
        order_n = np.argsort(-ideg, kind="stable")
        perms.append(c * NC + order_n)
    # Shared per-window column counts (max over cores, +1 self, KCOL-aligned).
    cws = np.zeros(NW, dtype=np.int64)
    for c in range(NCORES):
        s = indeg[perms[c]]
        pad = np.zeros(SLOTS, dtype=np.int64)
        pad[:NC] = s
        cws = np.maximum(cws, pad.reshape(NW, P).max(axis=1))
    cws = cws + 1                                    # self column
    cw_key = tuple(int(v) for v in cws)
    off = np.concatenate([[0], np.cumsum(cws)])
    CT = int(off[-1])

    nc0, nc1, nc2 = _get_kernels(cw_key)

    # ---------------- launch 0: broadcaster tables ----------------
    bcfT = np.zeros((P, MPAD), dtype=np.float32)
    bcfT[:, :M] = bc_feature.T
    a_pad = np.zeros(MPAD, dtype=np.int64)
    a_pad[:M] = assign[:M]
    selT = np.zeros((MPAD, MPAD), dtype=BF16NP)
    selT[a_pad[:M], np.arange(M)] = 1.0
    in0 = {
        "bcfT": bcfT, "WA": Walpha, "WZ": WZ,
        "dcnt": _pack_slots(cnt, np.float32(1.0), MCH),
        "selT": np.ascontiguousarray(
            selT.reshape(MCH, P, MPAD).transpose(1, 0, 2)
            .reshape(P, MCH * MPAD)
        ),
    }
    res0 = run_bass_kernel_spmd(nc0, [in0] * NCORES, core_ids=CORE_IDS)
    LAST_RESULTS.clear()
    LAST_RESULTS.append(res0)

    # chunk layout [128, 8*256]: row i lives at [i%128, (i//128)*256:...]
    def _unchunk(arr):
        return np.ascontiguousarray(
            arr.reshape(P, MCH, 2 * D).transpose(1, 0, 2).reshape(MPAD, 2 * D)
        )

    T_np = _unchunk(np.asarray(res0.results[0]["T"]))
    Tcomb = _unchunk(np.asarray(res0.results[0]["TC"]))

    iden = np.zeros((P, D), dtype=BF16NP)
    np.fill_diagonal(iden, 1.0)

    # ---------------- launch 1: node phase ----------------
    in_maps1 = []
    treps = []
    for c in range(NCORES):
        perm = perms[c]
        xpad = np.zeros((SLOTS, D), dtype=np.float32)
        xpad[:NC] = x[perm]
        trep = np.zeros((SLOTS, 2 * D), dtype=BF16NP)
        trep[:NC] = T_np[assign[perm]]
        eye_mask = perm < M
        if eye_mask.any():
            ranks = np.nonzero(eye_mask)[0]
            trep[ranks] = Tcomb[perm[ranks]]
        treps.append(trep)
        degv = np.ones(SLOTS, dtype=np.float32)
        degv[:NC] = deg[perm]
        in_maps1.append({
            "xT": np.ascontiguousarray(xpad.T),
            "WX": WX,
            "taT": np.ascontiguousarray(
                trep[:, :D].reshape(NW, P, D).transpose(1, 0, 2)
                .reshape(P, NW * D)
            ),
            "deg": np.ascontiguousarray(degv.reshape(NW, P).T),
            "ident": iden,
        })
    res1 = run_bass_kernel_spmd(nc1, in_maps1, core_ids=CORE_IDS)
    LAST_RESULTS.append(res1)

    # GS[p, w*D:] holds node perm[w*128+p]; restore node order globally.
    GSe = np.zeros((N + 1, D), dtype=BF16NP)   # +1 zero row for padding
    for c in range(NCORES):
        gs = np.asarray(res1.results[c]["GS"])       # [P, NW*D]
        gs = gs.reshape(P, NW, D).transpose(1, 0, 2).reshape(SLOTS, D)
        GSe[perms[c]] = gs[:NC]

    # ---------------- launch 2: edge phase ----------------
    in_maps2 = []
    for c in range(NCORES):
        perm = perms[c]
        slotof = np.empty(NC, dtype=np.int64)
        slotof[perm - c * NC] = np.arange(NC)
        lo, hi = bounds[c * NC], bounds[(c + 1) * NC]
        rnk = slotof[row_s[lo:hi] - c * NC]
        kth = np.arange(lo, hi) - bounds[row_s[lo:hi]]
        srcidx = np.full((P, CT), N, dtype=np.int64)
        # self column first, then the in-edges
        allrnk = np.arange(NC)
        srcidx[allrnk & 127, off[allrnk >> 7]] = perm
        srcidx[rnk & 127, off[rnk >> 7] + 1 + kth] = col_s[lo:hi]
        MSG = GSe[srcidx.ravel()].reshape(P, CT * D)
        degv = np.ones(SLOTS, dtype=np.float32)
        degv[:NC] = deg[perm]
        tzz = treps[c][:, D:]
        in_maps2.append({
            "MSG": np.ascontiguousarray(MSG),
            "Rw": np.ascontiguousarray(
                tzz.reshape(NW, P, D).transpose(1, 0, 2).reshape(P, NW * D)
            ),
            "degw": np.ascontiguousarray(degv.reshape(NW, P).T),
            "ident": iden,
        })
    res2 = run_bass_kernel_spmd(nc2, in_maps2, core_ids=CORE_IDS)
    LAST_RESULTS.append(res2)

    out = np.empty((N, D), dtype=np.float32)
    for c in range(NCORES):
        o = np.asarray(res2.results[c]["OUT"]).astype(np.float32)
        o = o.reshape(P, NW, D).transpose(1, 0, 2).reshape(SLOTS, D)
        out[perms[c]] = o[:NC]
    return out
